# revision 1
# baseline (speedup 1.0000x reference)
"""Trainium2 Bass kernel for nn_DecGreenNet_product_CP3.

Reference computation:
    lhs  = tanh(input @ Wx1 + bx1) @ Wx2 + bx2          # [B, 512]
    s_i  = sum_n sin(pi*eq*qx_n) * mlp_i(qx_n)           # [8,16] per branch
    rhs  = einsum('bx,dx,fx->bdf', s_a, s_c, s_e)        # [512]
    out  = lhs @ rhs                                     # [B]

Algebraic restructuring used here (validated to ~2e-6 rel err):
    out[b] = tanh(input[b] @ Wx1 + bx1) @ (Wx2 @ rhs) + bx2 @ rhs
    s      = W2^T @ (h1tanh^T @ y) + (sum y) * b2   per quad branch
collapsing the dominant [B,512]x[512,512] GEMM into a matvec.

Sharding: batch B split 8 ways (8192 rows/core); quad nodes split 8 ways
(1024 nodes/core) with a tiny [128,4] AllReduce of the per-core partial
s-vectors (the branch reduction is linear, so partials sum exactly).
"""

import numpy as np

import concourse.bacc as bacc
import concourse.bass as bass
import concourse.mybir as mybir
import concourse.tile as tile
from concourse.bass_utils import run_bass_kernel_spmd

F32 = mybir.dt.float32
F16 = mybir.dt.float16
AF = mybir.ActivationFunctionType
ALU = mybir.AluOpType

NCORES = 8
B, DIN, H = 65536, 3, 512
N, HQ = 8192, 128
S0, RX = 8, 16
BL = B // NCORES          # 8192 batch rows per core
NL = N // NCORES          # 1024 quad nodes per core
NT = NL // 128            # 8 node tiles per branch
CH = 512                  # batch chunk (columns per matmul)
NCH = BL // CH            # 16 chunks
HTILES = H // 128         # 4 h tiles

# scheduling knobs
EMIT_BEFORE = 16          # L1 chunks emitted before the post-collective block
HID_BUFS = 32             # keep all hidden tiles resident

# fp16 scaling: w values are ~1e10-1e11; scale into fp16 range (exact pow2)
RC_SCALE = 2.0 ** -36     # applied to rhs_vec before the fp16 w-matmuls
W_SCALE = 1.0             # applied on psum->sbuf copy of w (total 2^-36)
OUT_SCALE = 2.0 ** 36     # undo in the final output pass

# minimax odd polynomial for sin(t), t in [0, pi]: sin(t)=t*P(t^2), err<2e-5
SIN_C = (0.999984590176674, -0.16663258473611252, 8.312385898666645e-03,
         -1.9316230946716391e-04, 2.1732361127812407e-06)

_CACHED_NC = None

import os
_STAGE = os.environ.get("K_STAGE", "full")  # quad | cc | eins | mainonly | full


def _build():
    nc = bacc.Bacc("TRN2", target_bir_lowering=False, debug=False,
                   num_devices=NCORES)

    xT = nc.dram_tensor("xT", [DIN + 1, BL], F16, kind="ExternalInput").ap()
    wx1a = nc.dram_tensor("wx1a", [DIN + 1, H], F16, kind="ExternalInput").ap()
    wx2t = nc.dram_tensor("wx2tb", [64, 4096], F16, kind="ExternalInput").ap()
    bx2r = nc.dram_tensor("bx2rb", [64, 128], F16, kind="ExternalInput").ap()
    qxa = nc.dram_tensor("qxa", [6, NL], F16, kind="ExternalInput").ap()
    qxc = nc.dram_tensor("qxc", [128, 3 * NT], F32, kind="ExternalInput").ap()
    wqa = nc.dram_tensor("wqa", [6, HQ], F16, kind="ExternalInput").ap()
    wq2 = nc.dram_tensor("wq2", [HQ, 3 * HQ], F32, kind="ExternalInput").ap()
    bq2r = nc.dram_tensor("bq2r", [3, HQ], F32, kind="ExternalInput").ap()
    eqb = nc.dram_tensor("eqb", [128, 1], F32, kind="ExternalInput").ap()
    out_d = nc.dram_tensor("out", [BL], F32, kind="ExternalOutput").ap()

    global _APS
    _APS = (xT, wx1a, wx2t, bx2r, qxa, qxc, wqa, wq2, bq2r, eqb, out_d)
    with tile.TileContext(nc) as tc:
        _body(nc, tc)
    nc.compile()
    return nc


def _body(nc, tc):
        xT, wx1a, wx2t, bx2r, qxa, qxc, wqa, wq2, bq2r, eqb, out_d = _APS
        with (
            tc.tile_pool(name="const", bufs=1) as constp,
            tc.tile_pool(name="qsb", bufs=1) as qsb,
            tc.tile_pool(name="h1p", bufs=4) as h1p,
            tc.tile_pool(name="dram", bufs=2, space="DRAM") as dram,
            tc.tile_pool(name="tinyp", bufs=1, space="PSUM") as tinyp,
            tc.tile_pool(name="mainsb", bufs=1) as mainsb,
            tc.tile_pool(name="orowp", bufs=3) as orowp,
            tc.tile_pool(name="esb", bufs=2) as esb,
            tc.tile_pool(name="hidp", bufs=HID_BUFS) as hidp,
            tc.tile_pool(name="prep", bufs=2, space="PSUM") as prep,
            tc.tile_pool(name="outp", bufs=2, space="PSUM") as outpp,
        ):
            ones128 = constp.tile([128, 1], F32)
            nc.vector.memset(ones128, 1.0)

            # ---------------- quad phase DMAs ----------------
            # per-branch tiles so every matmul operand starts at partition 0
            # y-polynomial inputs first (critical path to the collective)
            qxc_sb = qsb.tile([128, 3 * NT], F32, tag="qxc")
            nc.sync.dma_start(out=qxc_sb, in_=qxc)
            eqb_sb = qsb.tile([128, 1], F32, tag="eqb")
            nc.sync.dma_start(out=eqb_sb, in_=eqb)
            qxa_sb, wqa_sb, bq2r_sb = [], [], []
            qeng = [nc.gpsimd, nc.sync, nc.gpsimd]
            for br in range(3):
                e = qeng[br]
                t = qsb.tile([2, NL], F16, tag=f"qxa{br}")
                e.dma_start(out=t, in_=qxa[2 * br:2 * br + 2, :])
                qxa_sb.append(t)
                t = qsb.tile([2, HQ], F16, tag=f"wqa{br}")
                e.dma_start(out=t, in_=wqa[2 * br:2 * br + 2, :])
                wqa_sb.append(t)
                t = qsb.tile([1, HQ], F32, tag=f"bq2r{br}")
                e.dma_start(out=t, in_=bq2r[br:br + 1, :])
                bq2r_sb.append(t)
            wq2_sb = qsb.tile([HQ, 3 * HQ], F32, tag="wq2")
            nc.sync.dma_start(out=wq2_sb, in_=wq2)

            qcut = int(os.environ.get("K_QCUT", "99"))

            def qdump(ap2d):
                p, c = ap2d.shape[0], ap2d.shape[1]
                nc.sync.dma_start(
                    out=out_d[0:p * c].rearrange("(p c) -> p c", c=c),
                    in_=ap2d)

            if qcut <= 1:
                qdump(qxc_sb[:, 0:3])
                return

            # y = sin(pi*eq*qx) via odd minimax polynomial on the DVE
            # (keeps ScalarE on a single act-table set: Tanh only)
            eqpi = qsb.tile([128, 1], F32, tag="eqpi")
            nc.vector.tensor_scalar_mul(eqpi, eqb_sb, float(np.pi))
            tq = qsb.tile([128, 3 * NT], F32, tag="tq")
            nc.vector.tensor_scalar_mul(tq, qxc_sb, eqpi[:, 0:1])
            t2 = qsb.tile([128, 3 * NT], F32, tag="t2")
            nc.vector.tensor_tensor(out=t2, in0=tq, in1=tq, op=ALU.mult)
            pp = qsb.tile([128, 3 * NT], F32, tag="pp")
            c1, c3, c5, c7, c9 = [float(v) for v in SIN_C]
            nc.vector.tensor_scalar(out=pp, in0=t2, scalar1=c9, scalar2=c7,
                                    op0=ALU.mult, op1=ALU.add)
            for cof in (c5, c3, c1):
                nc.vector.tensor_tensor(out=pp, in0=pp, in1=t2, op=ALU.mult)
                nc.vector.tensor_scalar_add(pp, pp, cof)
            y_sb = qsb.tile([128, 3 * NT], F16, tag="ysb")
            nc.vector.tensor_tensor(out=y_sb, in0=pp, in1=tq, op=ALU.mult)
            if qcut <= 2:
                qdump(y_sb[:, 0:3])
                return

            # ---------------- quad branches ----------------
            # qsmall columns: 0-2 = z per branch, 3-5 = sy per branch (row 0),
            # 6-8 = s per branch
            qsmall = tinyp.tile([128, 12], F32, tag="tiny")
            nc.vector.memset(qsmall[:, 3:6], 0.0)
            for br in range(3):
                h1s = []
                for half in range(2):
                    qpre = prep.tile([128, 512], F32, tag="pre")
                    for i2 in range(4):
                        i = half * 4 + i2
                        nc.tensor.matmul(
                            qpre[:, i2 * 128:(i2 + 1) * 128],
                            lhsT=qxa_sb[br][:, i * 128:(i + 1) * 128],
                            rhs=wqa_sb[br],
                            start=True, stop=True)
                    h1 = h1p.tile([128, 512], F16, tag="h1")
                    nc.scalar.activation(out=h1, in_=qpre, func=AF.Tanh)
                    h1s.append(h1)
                # z[h] = sum_n h1[n,h]*y[n], accumulated over 8 node tiles
                for i in range(NT):
                    nc.tensor.matmul(
                        qsmall[:, br:br + 1],
                        lhsT=h1s[i // 4][:, (i % 4) * 128:(i % 4 + 1) * 128],
                        rhs=y_sb[:, br * NT + i:br * NT + i + 1],
                        start=(i == 0), stop=(i == NT - 1))
                if qcut <= 5:
                    continue
                # sy = sum_n y[n]  -> row 0 of column 3+br
                ysum = qsb.tile([128, 1], F32, tag="ysum")
                nc.vector.tensor_reduce(
                    out=ysum, in_=y_sb[:, br * NT:(br + 1) * NT],
                    axis=mybir.AxisListType.X, op=ALU.add)
                nc.tensor.matmul(
                    qsmall[0:1, 3 + br:4 + br], lhsT=ysum[:, 0:1],
                    rhs=ones128[:, 0:1], start=True, stop=True)

            if qcut <= 3:
                qdump(h1s[0][:, 0:3])
                return

            z_sb = qsb.tile([128, 6], F32, tag="zsb")
            if qcut <= 5:
                nc.vector.tensor_copy(out=z_sb[:, 0:3], in_=qsmall[:, 0:3])
                qdump(z_sb[:, 0:3])
                return
            nc.vector.tensor_copy(out=z_sb, in_=qsmall[:, 0:6])
            if qcut <= 6:
                qdump(z_sb[:, 0:6])
                return
            # s = W2^T z + sy * b2 per branch -> columns 6..8
            for br in range(3):
                nc.tensor.matmul(
                    qsmall[:, 6 + br:7 + br],
                    lhsT=wq2_sb[:, br * HQ:(br + 1) * HQ],
                    rhs=z_sb[:, br:br + 1], start=True, stop=False)
                nc.tensor.matmul(
                    qsmall[:, 6 + br:7 + br],
                    lhsT=bq2r_sb[br],
                    rhs=z_sb[0:1, 3 + br:4 + br], start=False, stop=True)
            s_sb = qsb.tile([128, 3], F32, tag="ssb")
            nc.vector.tensor_copy(out=s_sb, in_=qsmall[:, 6:9])

            if _STAGE == "quad":
                nc.sync.dma_start(out=out_d[0:384],
                                  in_=s_sb.rearrange("p c -> (p c)"))
                return

            # ---------------- AllReduce of partial s ----------------
            # bounce buffers hold s already transposed to [16 x, (br, b)] so
            # the post-barrier read is a dense [16, 24] block
            cc_in = dram.tile([16, 24], F32, tag="ccin")
            cc_out = dram.tile([16, 24], F32, tag="ccout")
            nc.gpsimd.dma_start(out=cc_in.rearrange("x (c b) -> b x c", b=8),
                                in_=s_sb)
            nc.gpsimd.collective_compute(
                "AllReduce", ALU.add,
                replica_groups=[list(range(NCORES))],
                ins=[cc_in[:].opt()], outs=[cc_out[:].opt()])
            if _STAGE == "cc":
                sg_sb = qsb.tile([16, 24], F32, tag="sgsb")
                nc.gpsimd.dma_start(out=sg_sb, in_=cc_out)
                qdump(sg_sb)
                return

            # ---------------- main phase DMAs ----------------
            xT_sb = mainsb.tile([DIN + 1, BL], F16, tag="xT")
            nc.sync.dma_start(out=xT_sb, in_=xT)
            wx1a_sb = mainsb.tile([DIN + 1, H], F16, tag="wx1a")
            nc.sync.dma_start(out=wx1a_sb, in_=wx1a)
            wx2t_sb = mainsb.tile([64, 4096], F16, tag="wx2t")
            nc.sync.dma_start(out=wx2t_sb, in_=wx2t)
            bx2r_sb = mainsb.tile([64, 128], F16, tag="bx2r")
            nc.sync.dma_start(out=bx2r_sb, in_=bx2r)

            # ---------------- main L1 chunks (emitter) ----------------
            hid_tiles = {}

            def emit_l1(c):
                tiles = []
                for half in range(2):
                    pre = prep.tile([128, 1024], F32, tag="pre")
                    for k in range(2):
                        ht = half * 2 + k
                        nc.tensor.matmul(
                            pre[:, k * 512:(k + 1) * 512],
                            lhsT=wx1a_sb[:, ht * 128:(ht + 1) * 128],
                            rhs=xT_sb[:, c * CH:(c + 1) * CH],
                            start=True, stop=True)
                    hid = hidp.tile([128, 1024], F16, tag="hid")
                    nc.scalar.activation(out=hid, in_=pre, func=AF.Tanh)
                    tiles.append(hid)
                hid_tiles[c] = tiles

            for c in range(EMIT_BEFORE):
                emit_l1(c)

            # ---------------- post-collective small compute ----------------
            # s columns [128]=(b*16+x) -> sT[16x, (br,8b)] straight from the
            # collective's DRAM output (single strided DMA)
            sT_sb = esb.tile([16, 24], F32, tag="sT")
            nc.sync.dma_start(out=sT_sb, in_=cc_out)
            # E[x, d*8+f] = s_c[d,x] * s_e[f,x]
            sc_ap = sT_sb[:, 8:16]
            se_ap = sT_sb[:, 16:24]
            in0 = bass.AP(tensor=sc_ap.tensor, offset=sc_ap.offset,
                          ap=[sc_ap.ap[0], sc_ap.ap[1], [0, 8]])
            in1 = bass.AP(tensor=se_ap.tensor, offset=se_ap.offset,
                          ap=[se_ap.ap[0], [0, 8], se_ap.ap[1]])
            E_sb = esb.tile([16, 64], F32, tag="E")
            nc.vector.tensor_tensor(
                out=E_sb.rearrange("p (d f) -> p d f", f=8),
                in0=in0, in1=in1, op=ALU.mult)
            # rhs_vec[b, d*8+f] = sum_x s_a[b? -> see below] ;
            # out[b,df] = sum_x sT_a[x,b] * E[x,df]
            rhsp = tinyp.tile([64, 8], F32, tag="tiny")
            nc.tensor.matmul(rhsp, lhsT=E_sb, rhs=sT_sb[:, 0:8],
                             start=True, stop=True)
            r16 = esb.tile([64, 8], F16, tag="r16")
            nc.vector.tensor_scalar_mul(r16, rhsp, float(RC_SCALE))
            if _STAGE == "eins2":
                qdump(r16)
                return
            # w = Wx2 @ rhs_vec as [128, 4] (h = it*128+p), contracted over
            # b-blocks of 64 straight from the [64 df, 8 b] einsum layout
            wps = tinyp.tile([128, 4], F32, tag="tiny")
            for it in range(4):
                for b in range(8):
                    nc.tensor.matmul(
                        wps[:, it:it + 1],
                        lhsT=wx2t_sb[:, b * 512 + it * 128:b * 512 + (it + 1) * 128],
                        rhs=r16[:, b:b + 1],
                        start=(b == 0), stop=(b == 7))
            w_sb = esb.tile([128, 4], F16, tag="wsb")
            nc.vector.tensor_scalar_mul(w_sb, wps, float(W_SCALE))
            # c (scalar, scaled by RC_SCALE) replicated over 16 partitions
            c16p = tinyp.tile([16, 1], F32, tag="tiny")
            for b in range(8):
                nc.tensor.matmul(
                    c16p, lhsT=bx2r_sb[:, b * 16:(b + 1) * 16],
                    rhs=r16[:, b:b + 1],
                    start=(b == 0), stop=(b == 7))
            c16_sb = esb.tile([16, 1], F32, tag="c16")
            nc.vector.tensor_copy(out=c16_sb, in_=c16p)

            # ---------------- rest of L1 + dots ----------------
            def emit_dot(c):
                op = outpp.tile([1, 512], F32, tag="outp")
                for ht in range(HTILES):
                    nc.tensor.matmul(
                        op,
                        lhsT=w_sb[:, ht:ht + 1],
                        rhs=hid_tiles[c][ht // 2][:, (ht % 2) * 512:(ht % 2 + 1) * 512],
                        start=(ht == 0), stop=(ht == HTILES - 1))
                orow = orowp.tile([1, 512], F32, tag="outrow")
                nc.vector.tensor_scalar(
                    out=orow, in0=op, scalar1=c16_sb[0:1, 0:1],
                    scalar2=float(OUT_SCALE), op0=ALU.add, op1=ALU.mult)
                nc.sync.dma_start(
                    out=out_d[c * CH:(c + 1) * CH].rearrange("(o b) -> o b", o=1),
                    in_=orow)

            for c in range(EMIT_BEFORE, NCH):
                emit_l1(c)
            for c in range(NCH):
                emit_dot(c)


def _get_nc():
    global _CACHED_NC
    if _CACHED_NC is None:
        _CACHED_NC = _build()
    return _CACHED_NC


def _prep_in_maps(inputs):
    f = lambda k: np.ascontiguousarray(np.asarray(inputs[k], np.float32))
    inputx = f("input")
    eq = float(np.asarray(inputs["eq_param"]).reshape(-1)[0])
    Wx1, bx1 = f("Wx1"), f("bx1")
    Wx2, bx2 = f("Wx2"), f("bx2")

    wx1a = np.concatenate([Wx1, bx1[None, :]], axis=0).astype(np.float16)
    # wx2tb[df, b*512+it*128+i] = Wx2T[b*64+df, it*128+i]
    wx2tb = np.ascontiguousarray(
        Wx2.T.reshape(8, 64, 4, 128).transpose(1, 0, 2, 3).reshape(64, 4096)
    ).astype(np.float16)
    # bx2rb[df, b*16+m] = bx2[b*64+df]
    bx2rb = np.ascontiguousarray(
        np.repeat(bx2.reshape(8, 64).T[:, :, None], 16, axis=2).reshape(64, 128)
    ).astype(np.float16)
    wqa = np.empty((6, HQ), np.float16)
    bq2r = np.empty((3, HQ), np.float32)
    wq2 = np.empty((HQ, 3 * HQ), np.float32)
    qs = []
    for br, (qk, w1k, b1k, w2k, b2k) in enumerate([
            ("quad_x0", "Wq01", "bq01", "Wq02", "bq02"),
            ("quad_x1", "Wq11", "bq11", "Wq12", "bq12"),
            ("quad_x2", "Wq21", "bq21", "Wq22", "bq22")]):
        wqa[2 * br] = f(w1k)[0]
        wqa[2 * br + 1] = f(b1k)
        wq2[:, br * HQ:(br + 1) * HQ] = f(w2k)
        bq2r[br] = f(b2k)
        qs.append(f(qk)[:, 0])
    eqb = np.full((128, 1), eq, np.float32)

    shared = dict(wx1a=wx1a, wx2tb=wx2tb, bx2rb=bx2rb, wqa=wqa, wq2=wq2,
                  bq2r=bq2r, eqb=eqb)
    in_maps = []
    ones_row = np.ones((1, BL), np.float32)
    for c in range(NCORES):
        ish = inputx[c * BL:(c + 1) * BL]                        # [8192, 3]
        xTm = np.concatenate([ish.T, ones_row], axis=0)          # [4, 8192]
        qxa = np.empty((6, NL), np.float32)
        qxc = np.empty((128, 3 * NT), np.float32)
        for br in range(3):
            sh = qs[br][c * NL:(c + 1) * NL]
            qxa[2 * br] = sh
            qxa[2 * br + 1] = 1.0
            qxc[:, br * NT:(br + 1) * NT] = sh.reshape(NT, 128).T
        m = dict(shared)
        m["xT"] = np.ascontiguousarray(xTm).astype(np.float16)
        m["qxa"] = qxa.astype(np.float16)
        m["qxc"] = np.ascontiguousarray(qxc)
        in_maps.append(m)
    return in_maps


def _run(inputs, **kw):
    nc = _get_nc()
    in_maps = _prep_in_maps(inputs)
    res = run_bass_kernel_spmd(nc, in_maps, list(range(NCORES)), **kw)
    out = np.concatenate([res.results[c]["out"].reshape(-1)
                          for c in range(NCORES)]).astype(np.float32)
    return out, res


def kernel(**inputs) -> np.ndarray:
    out, _ = _run(inputs)
    return out


def kernel_traced(**inputs):
    """Correctness + NTFF profile (exec_time_ns) in one run."""
    return _run(inputs, trace=True)



# revision 10
# speedup vs baseline: 1.2628x; 1.2628x over previous
"""Trainium2 Bass kernel for nn_DecGreenNet_product_CP3.

Reference computation:
    lhs  = tanh(input @ Wx1 + bx1) @ Wx2 + bx2          # [B, 512]
    s_i  = sum_n sin(pi*eq*qx_n) * mlp_i(qx_n)           # [8,16] per branch
    rhs  = einsum('bx,dx,fx->bdf', s_a, s_c, s_e)        # [512]
    out  = lhs @ rhs                                     # [B]

Restructurings used here:
  1) out[b] = tanh(input[b] @ Wx1 + bx1) @ (Wx2 @ rhs) + bx2 @ rhs
     collapses the [B,512]x[512,512] GEMM into a matvec.
  2) The quad branch is a 1-D quadrature: z[h] = sum_n y_n tanh(w_h qx_n + b_h)
     with y = sin(pi*eq*qx).  Fit tanh(w_h x + b_h) on x in [0,1] with a
     degree-D polynomial in u = 2x-1 (coefficients from a G-point Chebyshev
     grid via a host-precomputed pseudoinverse), so
         z[h] ~= sum_j c_j[h] * m_j,   m_j = sum_n y_n u_n^j .
     The moments m_j are tiny DVE work; no per-node MLP is needed, so every
     core computes the full quadrature locally and NO collective is needed
     (validated to ~1e-3 final rel err in fp32 simulation).

Sharding: batch B split 8 ways (8192 rows/core); quadrature replicated.

Main loop: L1 hidden chunks ([128h,512b] x4 h-tiles, row-tiled into the PE
array at partitions 0/32/64/96 so the 4 matmuls run concurrently), tanh on
ScalarE (the bottleneck engine, kept saturated), dot chunks interleaved in
PE program order, per-pair output scaling on GpSimd, fp16 matvec with
2^-36/2^36 scaling as in the validated baseline numerics.
"""

import numpy as np

import concourse.bacc as bacc
import concourse.bass as bass
import concourse.mybir as mybir
import concourse.tile as tile
from concourse.bass_utils import run_bass_kernel_spmd

F32 = mybir.dt.float32
F16 = mybir.dt.float16
AF = mybir.ActivationFunctionType
ALU = mybir.AluOpType
AX = mybir.AxisListType

NCORES = 8
B, DIN, H = 65536, 3, 512
N, HQ = 8192, 128
S0, RX = 8, 16
BL = B // NCORES          # 8192 batch rows per core
CH = 512                  # batch chunk (columns per matmul)
NCH = BL // CH            # 16 chunks
NT = N // 128             # 64 node columns (full quadrature per core)

PDEG = 10                 # poly degree in u = 2*qx-1
NC_ = PDEG + 1            # coefficients per branch
G = 64                    # tanh fit grid size

# fp16 scaling for the matvec chain (w ~ 1e10..1e11)
RC_SCALE = 2.0 ** -36
OUT_SCALE = 2.0 ** 36

# minimax odd polynomial for sin(t), t in [0, pi]: sin(t)=t*P(t^2), err<2e-5
SIN_C = (0.999984590176674, -0.16663258473611252, 8.312385898666645e-03,
         -1.9316230946716391e-04, 2.1732361127812407e-06)

DOT_LAG = 8               # dot(c-DOT_LAG) emitted after L1(c)

_CACHED_NC = None

import os
_STAGE = os.environ.get("K_STAGE", "full")   # m | s | eins | full


def _host_grid_P():
    """Chebyshev grid on [0,1] and pinv mapping grid samples -> monomial
    coefficients in u = 2x-1 (host float64, cast to fp32)."""
    g = np.cos((2 * np.arange(G) + 1) / (2 * G) * np.pi)     # (-1,1)
    xg = (g + 1.0) / 2.0                                     # (0,1)
    u = 2.0 * xg - 1.0
    V = np.stack([u ** j for j in range(NC_)], axis=1)       # [G, NC_]
    P = np.linalg.pinv(V)                                    # [NC_, G]
    return xg, P


_XG, _P = _host_grid_P()


def _build():
    nc = bacc.Bacc("TRN2", target_bir_lowering=False, debug=False,
                   num_devices=NCORES)

    xT = nc.dram_tensor("xT", [DIN + 1, BL], F16, kind="ExternalInput").ap()
    wx1t = nc.dram_tensor("wx1t", [128, 128], F16, kind="ExternalInput").ap()
    wx2t = nc.dram_tensor("wx2tb", [64, 4096], F16, kind="ExternalInput").ap()
    bx2r = nc.dram_tensor("bx2rb", [64, 128], F16, kind="ExternalInput").ap()
    qxc = nc.dram_tensor("qxc", [128, 3 * NT], F32, kind="ExternalInput").ap()
    wqa = nc.dram_tensor("wqa", [6, HQ], F16, kind="ExternalInput").ap()
    wq2 = nc.dram_tensor("wq2", [HQ, 3 * HQ], F32, kind="ExternalInput").ap()
    bq2r = nc.dram_tensor("bq2r", [3, HQ], F32, kind="ExternalInput").ap()
    eqb = nc.dram_tensor("eqb", [128, 1], F32, kind="ExternalInput").ap()
    gx2 = nc.dram_tensor("gx2", [2, G], F16, kind="ExternalInput").ap()
    ptm = nc.dram_tensor("ptm", [G, NC_], F32, kind="ExternalInput").ap()
    out_d = nc.dram_tensor("out", [BL], F32, kind="ExternalOutput").ap()

    global _APS
    _APS = (xT, wx1t, wx2t, bx2r, qxc, wqa, wq2, bq2r, eqb, gx2, ptm, out_d)
    with tile.TileContext(nc) as tc:
        _body(nc, tc)
    nc.compile()
    return nc


def _body(nc, tc):
    xT, wx1t, wx2t, bx2r, qxc, wqa, wq2, bq2r, eqb, gx2, ptm, out_d = _APS
    with (
        tc.tile_pool(name="const", bufs=1) as constp,
        tc.tile_pool(name="qsb", bufs=1) as qsb,
        tc.tile_pool(name="dram", bufs=1, space="DRAM") as dram,
        tc.tile_pool(name="mainsb", bufs=1) as mainsb,
        tc.tile_pool(name="hidp", bufs=10) as hidp,
        tc.tile_pool(name="orowp", bufs=2) as orowp,
        tc.tile_pool(name="prep", bufs=2, space="PSUM") as prep,
        tc.tile_pool(name="quadp", bufs=2, space="PSUM") as quadp,
        tc.tile_pool(name="outp", bufs=1, space="PSUM") as outpp,
    ):
        ones128 = constp.tile([128, 1], F32)
        nc.vector.memset(ones128, 1.0)

        # ---------------- DMAs: quad-critical first ----------------
        qxc_sb = qsb.tile([128, 3 * NT], F32, tag="qxc")
        nc.sync.dma_start(out=qxc_sb, in_=qxc)
        eqb_sb = qsb.tile([128, 1], F32, tag="eqb")
        nc.sync.dma_start(out=eqb_sb, in_=eqb)
        gx2_sb = qsb.tile([2, G], F16, tag="gx2")
        nc.gpsimd.dma_start(out=gx2_sb, in_=gx2)
        wqa_sb = []
        for br in range(3):
            t = qsb.tile([2, HQ], F16, tag=f"wqa{br}")
            nc.gpsimd.dma_start(out=t, in_=wqa[2 * br:2 * br + 2, :])
            wqa_sb.append(t)
        ptm_sb = qsb.tile([G, NC_], F32, tag="ptm")
        nc.gpsimd.dma_start(out=ptm_sb, in_=ptm)

        # main-phase DMAs (needed from ~13us on)
        wx1t_sb = mainsb.tile([128, 128], F16, tag="wx1t")
        nc.gpsimd.dma_start(out=wx1t_sb, in_=wx1t)
        xT4_sb = mainsb.tile([128, BL], F16, tag="xT4")
        for i in range(4):
            nc.gpsimd.dma_start(out=xT4_sb[32 * i:32 * i + 4, :], in_=xT)

        # quad layer-2 + einsum weights (needed ~22us on)
        wq2_sb = qsb.tile([HQ, 3 * HQ], F32, tag="wq2")
        nc.sync.dma_start(out=wq2_sb, in_=wq2)
        bq2r_sb = []
        for br in range(3):
            t = qsb.tile([1, HQ], F32, tag=f"bq2r{br}")
            nc.sync.dma_start(out=t, in_=bq2r[br:br + 1, :])
            bq2r_sb.append(t)
        wx2t_sb = mainsb.tile([64, 4096], F16, tag="wx2t")
        nc.gpsimd.dma_start(out=wx2t_sb, in_=wx2t)
        bx2r_sb = mainsb.tile([64, 128], F16, tag="bx2r")
        nc.gpsimd.dma_start(out=bx2r_sb, in_=bx2r)

        # ---------------- quad: tanh grid eval (PE+ACT, early) ----------
        tg_sb = []
        for br in range(3):
            tg_ps = quadp.tile([G, HQ], F32, tag="qp")
            nc.tensor.matmul(tg_ps, lhsT=gx2_sb, rhs=wqa_sb[br],
                             start=True, stop=True)
            t_sb = qsb.tile([G, HQ], F32, tag=f"tsb{br}")
            nc.scalar.activation(out=t_sb, in_=tg_ps, func=AF.Tanh)
            tg_sb.append(t_sb)

        # ---------------- quad: y = sin(pi*eq*qx), moments on DVE --------
        eqpi = qsb.tile([128, 1], F32, tag="eqpi")
        nc.vector.tensor_scalar_mul(eqpi, eqb_sb, float(np.pi))
        tq = qsb.tile([128, 3 * NT], F32, tag="tq")
        nc.vector.tensor_scalar_mul(tq, qxc_sb, eqpi[:, 0:1])
        t2 = qsb.tile([128, 3 * NT], F32, tag="t2")
        nc.vector.tensor_tensor(out=t2, in0=tq, in1=tq, op=ALU.mult)
        pp = qsb.tile([128, 3 * NT], F32, tag="pp")
        c1, c3, c5, c7, c9 = [float(v) for v in SIN_C]
        nc.vector.tensor_scalar(out=pp, in0=t2, scalar1=c9, scalar2=c7,
                                op0=ALU.mult, op1=ALU.add)
        for cof in (c5, c3, c1):
            nc.vector.tensor_tensor(out=pp, in0=pp, in1=t2, op=ALU.mult)
            nc.vector.tensor_scalar_add(pp, pp, cof)
        # u = 2*qx - 1
        u_sb = qsb.tile([128, 3 * NT], F32, tag="usb")
        nc.vector.tensor_scalar(out=u_sb, in0=qxc_sb, scalar1=2.0,
                                scalar2=-1.0, op0=ALU.mult, op1=ALU.add)
        # stack[:, j, br, t] = y * u^j  (chain); y into j=0 slot
        stack = qsb.tile([128, NC_ * 3 * NT], F32, tag="stack")
        stk = stack.rearrange("p (j c t) -> p j c t", j=NC_, t=NT)
        nc.vector.tensor_tensor(out=stk[:, 0, :, :],
                                in0=pp, in1=tq, op=ALU.mult)
        u3 = u_sb.rearrange("p (c t) -> p c t", t=NT)
        for j in range(1, NC_):
            nc.vector.tensor_tensor(out=stk[:, j, :, :],
                                    in0=stk[:, j - 1, :, :], in1=u3,
                                    op=ALU.mult)
        red = qsb.tile([128, NC_ * 3], F32, tag="red")
        nc.vector.tensor_reduce(out=red, in_=stk, axis=AX.X, op=ALU.add)
        red3 = red.rearrange("p (j c) -> p j c", c=3)

        # ---------------- main L1 chunks (row-tiled 4-pack) --------------
        hid_tiles = {}

        def emit_l1(c):
            hid = hidp.tile([128, 4 * CH], F16, tag="hid")
            for half in range(2):
                pre = prep.tile([128, 2 * CH], F32, tag="pre")
                for k in range(2):
                    ht = half * 2 + k
                    bp = 32 * ht
                    nc.tensor.matmul(
                        pre[:, k * CH:(k + 1) * CH],
                        lhsT=wx1t_sb[bp:bp + 4, :],
                        rhs=xT4_sb[bp:bp + 4, c * CH:(c + 1) * CH],
                        start=True, stop=True,
                        tile_position=(bp, 0))
                nc.scalar.activation(
                    out=hid[:, half * 2 * CH:(half + 1) * 2 * CH],
                    in_=pre, func=AF.Tanh)
            hid_tiles[c] = hid

        L1_BEFORE_QUAD = 6
        for c in range(L1_BEFORE_QUAD):
            emit_l1(c)

        # ---------------- quad: m -> c -> z -> s (tiny MMs) --------------
        m_ps = quadp.tile([NC_, 3], F32, tag="qp")
        for br in range(3):
            nc.tensor.matmul(m_ps[:, br:br + 1], lhsT=red3[:, :, br],
                             rhs=ones128, start=True, stop=True)
        m_sb = qsb.tile([NC_, 3], F32, tag="msb")
        nc.vector.tensor_copy(out=m_sb, in_=m_ps)
        if _STAGE == "m":
            nc.sync.dma_start(out=out_d[0:NC_ * 3]
                              .rearrange("(p c) -> p c", c=3), in_=m_sb)
            return

        c_sb = []
        for br in range(3):
            c_ps = quadp.tile([NC_, HQ], F32, tag="qp")
            nc.tensor.matmul(c_ps, lhsT=ptm_sb, rhs=tg_sb[br],
                             start=True, stop=True)
            cs = qsb.tile([NC_, HQ], F32, tag=f"csb{br}")
            nc.vector.tensor_copy(out=cs, in_=c_ps)
            c_sb.append(cs)
        z_ps = quadp.tile([128, 3], F32, tag="qp")
        for br in range(3):
            nc.tensor.matmul(z_ps[:, br:br + 1], lhsT=c_sb[br],
                             rhs=m_sb[:, br:br + 1],
                             start=True, stop=True)
        z_sb = qsb.tile([128, 3], F32, tag="zsb")
        nc.vector.tensor_copy(out=z_sb, in_=z_ps)
        s_ps = quadp.tile([128, 3], F32, tag="qp")
        for br in range(3):
            nc.tensor.matmul(s_ps[:, br:br + 1],
                             lhsT=wq2_sb[:, br * HQ:(br + 1) * HQ],
                             rhs=z_sb[:, br:br + 1], start=True, stop=False)
            nc.tensor.matmul(s_ps[:, br:br + 1],
                             lhsT=bq2r_sb[br],
                             rhs=m_sb[0:1, br:br + 1],
                             start=False, stop=True)
        s_sb = qsb.tile([128, 3], F32, tag="ssb")
        nc.vector.tensor_copy(out=s_sb, in_=s_ps)
        if _STAGE == "s":
            nc.sync.dma_start(out=out_d[0:384],
                              in_=s_sb.rearrange("p c -> (p c)"))
            return

        # transpose bounce: s[(b,x), br] -> sT[x, (br, b)] via DRAM
        bounce = dram.tile([16, 24], F32, tag="bounce")
        nc.gpsimd.dma_start(out=bounce.rearrange("x (c b) -> b x c", b=8),
                            in_=s_sb)

        emit_l1(L1_BEFORE_QUAD)      # c = 6

        # ---------------- einsum + w chain ----------------
        sT_sb = qsb.tile([16, 24], F32, tag="sT")
        nc.sync.dma_start(out=sT_sb, in_=bounce)
        sc_ap = sT_sb[:, 8:16]
        se_ap = sT_sb[:, 16:24]
        in0 = bass.AP(tensor=sc_ap.tensor, offset=sc_ap.offset,
                      ap=[sc_ap.ap[0], sc_ap.ap[1], [0, 8]])
        in1 = bass.AP(tensor=se_ap.tensor, offset=se_ap.offset,
                      ap=[se_ap.ap[0], [0, 8], se_ap.ap[1]])
        E_sb = qsb.tile([16, 64], F32, tag="E")
        nc.vector.tensor_tensor(
            out=E_sb.rearrange("p (d f) -> p d f", f=8),
            in0=in0, in1=in1, op=ALU.mult)
        rhsp = quadp.tile([64, 8], F32, tag="qp")
        nc.tensor.matmul(rhsp, lhsT=E_sb, rhs=sT_sb[:, 0:8],
                         start=True, stop=True)
        r16 = qsb.tile([64, 8], F16, tag="r16")
        nc.vector.tensor_scalar_mul(r16, rhsp, float(RC_SCALE))
        if _STAGE == "eins":
            nc.sync.dma_start(
                out=out_d[0:512].rearrange("(p c) -> p c", c=8), in_=r16)
            return

        emit_l1(L1_BEFORE_QUAD + 1)  # c = 7

        # w = Wx2 @ rhs_vec as [128, 4] fp16 (scaled by 2^-36)
        wps = quadp.tile([128, 4], F32, tag="qp")
        for it in range(4):
            for b in range(8):
                nc.tensor.matmul(
                    wps[:, it:it + 1],
                    lhsT=wx2t_sb[:, b * 512 + it * 128:b * 512 + (it + 1) * 128],
                    rhs=r16[:, b:b + 1],
                    start=(b == 0), stop=(b == 7))
        w_sb = qsb.tile([128, 4], F16, tag="wsb")
        nc.vector.tensor_copy(out=w_sb, in_=wps)
        # c scalar (scaled), replicated over 16 partitions; use [0,0]
        c16p = quadp.tile([16, 1], F32, tag="qp")
        for b in range(8):
            nc.tensor.matmul(
                c16p, lhsT=bx2r_sb[:, b * 16:(b + 1) * 16],
                rhs=r16[:, b:b + 1],
                start=(b == 0), stop=(b == 7))
        c16_sb = qsb.tile([16, 1], F32, tag="c16")
        nc.vector.tensor_copy(out=c16_sb, in_=c16p)

        # ---------------- dots interleaved with remaining L1 -------------
        pair_ps = {}

        def emit_dot(c):
            p, off = c // 2, (c % 2) * CH
            if c % 2 == 0:
                pair_ps[p] = outpp.tile([1, 2 * CH], F32, tag="op",
                                        name=f"op{p}")
            op = pair_ps[p]
            for ht in range(4):
                nc.tensor.matmul(
                    op[0:1, off:off + CH],
                    lhsT=w_sb[:, ht:ht + 1],
                    rhs=hid_tiles[c][:, ht * CH:(ht + 1) * CH],
                    start=(ht == 0), stop=(ht == 3))
            if c % 2 == 1:
                orow = orowp.tile([1, 2 * CH], F32, tag="orow")
                nc.vector.tensor_scalar(
                    out=orow, in0=op, scalar1=c16_sb[0:1, 0:1],
                    scalar2=float(OUT_SCALE), op0=ALU.add, op1=ALU.mult)
                nc.sync.dma_start(
                    out=out_d[(p * 2) * CH:(p * 2 + 2) * CH]
                        .rearrange("(o b) -> o b", o=1),
                    in_=orow)

        for c in range(L1_BEFORE_QUAD + 2, NCH):
            emit_l1(c)
            if c >= DOT_LAG:
                emit_dot(c - DOT_LAG)
        for c in range(NCH - DOT_LAG, NCH):
            emit_dot(c)


def _get_nc():
    global _CACHED_NC
    if _CACHED_NC is None:
        _CACHED_NC = _build()
    return _CACHED_NC


def _prep_in_maps(inputs):
    f = lambda k: np.ascontiguousarray(np.asarray(inputs[k], np.float32))
    inputx = f("input")
    eq = float(np.asarray(inputs["eq_param"]).reshape(-1)[0])
    Wx1, bx1 = f("Wx1"), f("bx1")
    Wx2, bx2 = f("Wx2"), f("bx2")

    # wx1t: row-tiled L1 weights: partition 32i+k = (Wx1 row k | bx1)[h-tile i]
    wx1t = np.zeros((128, 128), np.float16)
    wx1a = np.concatenate([Wx1, bx1[None, :]], axis=0)       # [4, 512]
    for i in range(4):
        wx1t[32 * i:32 * i + 4, :] = wx1a[:, i * 128:(i + 1) * 128]
    # wx2tb[df, b*512+it*128+i] = Wx2T[b*64+df, it*128+i]
    wx2tb = np.ascontiguousarray(
        Wx2.T.reshape(8, 64, 4, 128).transpose(1, 0, 2, 3).reshape(64, 4096)
    ).astype(np.float16)
    # bx2rb[df, b*16+m] = bx2[b*64+df]
    bx2rb = np.ascontiguousarray(
        np.repeat(bx2.reshape(8, 64).T[:, :, None], 16, axis=2).reshape(64, 128)
    ).astype(np.float16)
    wqa = np.empty((6, HQ), np.float16)
    bq2r = np.empty((3, HQ), np.float32)
    wq2 = np.empty((HQ, 3 * HQ), np.float32)
    qxc = np.empty((128, 3 * NT), np.float32)
    for br, (qk, w1k, b1k, w2k, b2k) in enumerate([
            ("quad_x0", "Wq01", "bq01", "Wq02", "bq02"),
            ("quad_x1", "Wq11", "bq11", "Wq12", "bq12"),
            ("quad_x2", "Wq21", "bq21", "Wq22", "bq22")]):
        wqa[2 * br] = f(w1k)[0]
        wqa[2 * br + 1] = f(b1k)
        wq2[:, br * HQ:(br + 1) * HQ] = f(w2k)
        bq2r[br] = f(b2k)
        qxc[:, br * NT:(br + 1) * NT] = f(qk)[:, 0].reshape(NT, 128).T
    eqb = np.full((128, 1), eq, np.float32)
    gx2 = np.stack([_XG, np.ones(G)], axis=0).astype(np.float16)  # [2, G]
    ptm = np.ascontiguousarray(_P.T).astype(np.float32)           # [G, NC_]

    shared = dict(wx1t=wx1t, wx2tb=wx2tb, bx2rb=bx2rb, wqa=wqa, wq2=wq2,
                  bq2r=bq2r, eqb=eqb, qxc=qxc, gx2=gx2, ptm=ptm)
    in_maps = []
    ones_row = np.ones((1, BL), np.float32)
    for c in range(NCORES):
        ish = inputx[c * BL:(c + 1) * BL]                        # [8192, 3]
        xTm = np.concatenate([ish.T, ones_row], axis=0)          # [4, 8192]
        m = dict(shared)
        m["xT"] = np.ascontiguousarray(xTm).astype(np.float16)
        in_maps.append(m)
    return in_maps


def _run(inputs, **kw):
    nc = _get_nc()
    in_maps = _prep_in_maps(inputs)
    res = run_bass_kernel_spmd(nc, in_maps, list(range(NCORES)), **kw)
    out = np.concatenate([res.results[c]["out"].reshape(-1)
                          for c in range(NCORES)]).astype(np.float32)
    return out, res


def kernel(**inputs) -> np.ndarray:
    out, _ = _run(inputs)
    return out


def kernel_traced(**inputs):
    """Correctness + NTFF profile (exec_time_ns) in one run."""
    return _run(inputs, trace=True)
